# revision 19
# baseline (speedup 1.0000x reference)
import numpy as np

B, S, D, H = 16, 4096, 256, 256
NCORES = 8
BLOCAL = B // NCORES  # 2

_CACHE = {}


def _build(C=128, W=6, wdt_name="bfloat16", has_bias=False, has_h0=False):
    """Chunked-restart RNN scan, one core, B_local=2, two half-pipelines.

    The tanh recurrence forgets its state in ~32 steps (contractive), so each
    sequence is split into C chunks that advance in PARALLEL as matmul
    columns, with W warmup steps per chunk (chunk 0 exact). NM = S/C + W
    macro-steps replace S serial steps.

    The 2C columns are further split into two independent HALF-PIPELINES
    (chunks 0..C/2-1 and C/2..C-1) with separate PSUM banks and ht tiles:
    half X's matmuls execute while half Y's tanh runs, so the serial
    matmul->psum->tanh->sbuf round trip of one half hides behind the other.
    Per half and step: 4 quadrant matmuls ([128,128] lhsT x [128,HC] rhs,
    accumulating on the prefilled x-projection) and one Tanh [128, 2, HC].
    The xp GEMM for block n+1 is emitted piecewise before each step's scan
    matmuls (ready at emission, it fills tanh-idle PE slots).
    """
    import concourse.bass as bass
    import concourse.tile as tile
    from concourse import bacc, mybir

    f32 = mybir.dt.float32
    wdt = getattr(mybir.dt, wdt_name)
    L = S // C
    NM = L + W
    CC = 2 * C
    HC = CC // 2  # columns per half-pipeline
    SB = 256 // HC  # macro-steps per block: [128,2,SB,HC] f32 = one 2KB bank
    assert SB >= 1 and NM % SB == 0 and W % SB == 0
    NBLK = NM // SB
    Tanh = mybir.ActivationFunctionType.Tanh
    PSUM = bass.MemorySpace.PSUM

    nc = bacc.Bacc("TRN2", target_bir_lowering=False, debug=False)
    xs_d = nc.dram_tensor("xs", [D, NM, 2, HC], wdt, kind="ExternalInput")
    wx_d = nc.dram_tensor("wx", [D, H], wdt, kind="ExternalInput")
    wh_d = nc.dram_tensor("wh", [H, H], wdt, kind="ExternalInput")
    if has_bias:
        bias_d = nc.dram_tensor("bias", [1, H], wdt, kind="ExternalInput")
    if has_h0:
        hcorr_d = nc.dram_tensor("hcorr", [128, 2, 2], wdt, kind="ExternalInput")
    yt_d = nc.dram_tensor("yt", [128, 2, NM, 2, HC], wdt, kind="ExternalOutput")

    with tile.TileContext(nc) as tc:
        frees = []

        def T(shape, dt, name, space=None):
            kw = {"space": space} if space is not None else {}
            t, f = tc.tile(shape, dt, name=name, **kw)
            frees.append(f)
            return t

        wx_sb = T([128, 2, H], wdt, "wx_sb")
        scr_sb = T([128, 2], wdt, "scr_sb")
        wh_sb = T([128, 2, H], wdt, "wh_sb")
        h00_sb = T([128, 2, 2, HC], wdt, "h00_sb")
        xs_sb = [T([128, 2, SB, 2, HC], wdt, f"xs{i}") for i in range(2)]
        ht_sb = [
            [T([128, 2, SB, HC], wdt, f"ht{r}a{a}") for a in range(2)]
            for r in range(4)
        ]
        banks = [
            [T([128, 2, SB, HC], f32, f"pb{r}a{a}", space=PSUM) for a in range(2)]
            for r in range(4)
        ]
        if has_bias:
            bias_sb = T([1, H], wdt, "bias_sb")
            ones_sb = T([1, SB, 2, HC], wdt, "ones_sb")
            nc.sync.dma_start(bias_sb[:, :], bias_d[:, :])
        if has_h0:
            hcorr_sb = T([128, 2, 2], wdt, "hcorr_sb")
            nc.sync.dma_start(hcorr_sb[:, :, :], hcorr_d[:, :, :])

        def dma_in(blk, engines=None):
            if engines is None:
                engines = [nc.sync]
            for k in range(2):
                eng = engines[k % len(engines)]
                eng.dma_start(
                    xs_sb[blk % 2][:, k, :, :, :],
                    xs_d[k * 128 : (k + 1) * 128, blk * SB : (blk + 1) * SB, :, :],
                )

        def ones_for(blk):
            # bias applies everywhere except chunk 0's warmup columns (must
            # stay exactly zero; chunk 0 lives in half 0, column 0/1)
            nc.gpsimd.memset(ones_sb[:, :, :, :], 1.0)
            if blk * SB < W:
                nc.gpsimd.memset(ones_sb[0:1, :, 0, 0:2], 0.0)

        def gemm_mm(blk, ha, g):
            # g-th GEMM piece (of 4: m0k0, m0k1, m1k0, m1k1) prefilling block
            # blk's xp for half ha. m0k0 opens the bank's accumulation group
            # (start=True wipes the whole bank, so it must come first).
            m, k = g >> 1, g & 1
            nc.tensor.matmul(
                banks[blk % 4][ha][:, m, :, :],
                wx_sb[:, k, m * 128 : (m + 1) * 128],
                xs_sb[blk % 2][:, k, :, ha, :],
                start=(g == 0),
                stop=False,
                skip_group_check=True,
            )
            if has_bias and k == 1:
                nc.tensor.matmul(
                    banks[blk % 4][ha][:, m, :, :],
                    bias_sb[:, m * 128 : (m + 1) * 128],
                    ones_sb[:, :, ha, :],
                    start=False,
                    stop=False,
                    skip_group_check=True,
                )

        # prologue: block-0/1 xs first (gates the first GEMM pieces), then
        # weights split across queues; block 0's GEMM is emitted just-in-time
        # inside its first step below.
        # Scalar queue does ONLY the tanh-table preload (1283ns) so the
        # first real TANH doesn't pay it; weights go on sync/gpsimd after
        # the block-0 xs DMAs that gate the first GEMM pieces.
        nc.scalar.memzero(scr_sb[:, :])
        nc.scalar.activation(
            scr_sb[:, :], scr_sb[:, :], Tanh, bias=0.0, scale=1.0
        )
        dma_in(0, engines=[nc.sync, nc.gpsimd])
        nc.sync.dma_start(wx_sb[:, 0, :], wx_d[0:128, :])
        nc.sync.dma_start(wx_sb[:, 1, :], wx_d[128:256, :])
        nc.gpsimd.dma_start(wh_sb[:, 0, :], wh_d[0:128, :])
        nc.gpsimd.dma_start(wh_sb[:, 1, :], wh_d[128:256, :])
        dma_in(1)
        nc.gpsimd.memset(h00_sb[:, :, :, :], 0.0)
        if has_bias:
            ones_for(0)

        for blk in range(NBLK):
            hb = blk % 4
            if blk + 2 < NBLK and blk > 0:
                dma_in(blk + 2)
            if has_bias and blk + 1 < NBLK:
                ones_for(blk + 1)
            for j in range(SB):
                i = blk * SB + j
                for ha in range(2):
                    if blk == 0 and j == 0:
                        for g in range(4):
                            gemm_mm(0, ha, g)
                        if ha == 1 and NBLK > 2:
                            dma_in(2)  # all block-0 xs reads emitted by now
                    # next block's GEMM pieces: ready at emission, they run
                    # inside the other half's tanh window and keep PE warm
                    if blk + 1 < NBLK:
                        for g in range((4 * j) // SB, (4 * (j + 1)) // SB):
                            gemm_mm(blk + 1, ha, g)
                    if j > 0:
                        hp = ht_sb[hb][ha]
                        pj = j - 1
                    elif blk > 0:
                        hp = ht_sb[(hb - 1) % 4][ha]
                        pj = SB - 1
                    else:
                        hp = None
                    for k in range(2):
                        for m in range(2):
                            nc.tensor.matmul(
                                banks[hb][ha][:, m, j, :],
                                wh_sb[:, k, m * 128 : (m + 1) * 128],
                                h00_sb[:, k, ha, :] if hp is None else hp[:, k, pj, :],
                                start=False,
                                stop=(k == 1 and j == SB - 1),
                                skip_group_check=True,
                            )
                    if has_h0 and i == W and ha == 0:
                        # inject state0 @ Wh into chunk 0's first real column
                        for m in range(2):
                            for k in range(2):
                                nc.tensor.matmul(
                                    banks[hb][0][:, m, j, 0:2],
                                    wh_sb[:, k, m * 128 : (m + 1) * 128],
                                    hcorr_sb[:, k, :],
                                    start=False,
                                    stop=False,
                                    skip_group_check=True,
                                )
                    nc.scalar.activation(
                        ht_sb[hb][ha][:, :, j, :],
                        banks[hb][ha][:, :, j, :],
                        Tanh,
                        bias=0.0,
                        scale=1.0,
                    )
            if (blk + 1) * SB > W:  # warmup-only blocks are never read back
                for ha in range(2):
                    nc.gpsimd.dma_start(
                        yt_d[:, :, blk * SB : (blk + 1) * SB, ha, :],
                        ht_sb[hb][ha][:, :, :, :],
                    )

        for f in reversed(frees):
            f()

    nc.compile()
    return nc


def _get_nc(C, W, wdt_name, has_bias, has_h0):
    key = (C, W, wdt_name, has_bias, has_h0)
    if key not in _CACHE:
        _CACHE[key] = _build(C, W, wdt_name, has_bias, has_h0)
    return _CACHE[key]


LAST_EXEC_NS = None
LAST_RESULTS = None


def _np_dt(wdt_name):
    if wdt_name == "bfloat16":
        import ml_dtypes

        return ml_dtypes.bfloat16
    if wdt_name == "float16":
        return np.float16
    return np.float32


def kernel(inputs, state0, Wx, Wh, b, C=32, W=64, wdt_name="bfloat16", trace=False):
    global LAST_EXEC_NS, LAST_RESULTS
    from concourse.bass_utils import run_bass_kernel_spmd

    inputs = np.asarray(inputs, dtype=np.float32)
    state0 = np.asarray(state0, dtype=np.float32)
    Wx = np.asarray(Wx, dtype=np.float32)
    Wh = np.asarray(Wh, dtype=np.float32)
    b = np.asarray(b, dtype=np.float32)
    has_bias = bool(np.any(b != 0))
    has_h0 = bool(np.any(state0 != 0))
    ndt = _np_dt(wdt_name)
    L = S // C
    NM = L + W
    CC = 2 * C

    nc = _get_nc(C, W, wdt_name, has_bias, has_h0)

    wx_c = np.ascontiguousarray(Wx, dtype=ndt)
    wh_c = np.ascontiguousarray(Wh, dtype=ndt)

    # schedule gather indices: macro i, chunk c -> global step c*L - W + i
    ii = np.arange(NM)[:, None]
    cc_ = np.arange(C)[None, :]
    g = cc_ * L - W + ii  # [NM, C]
    valid = g >= 0
    gc = np.clip(g, 0, S - 1)

    in_maps = []
    for core in range(NCORES):
        xc = inputs[BLOCAL * core : BLOCAL * (core + 1)]  # [2, S, D]
        # xsched[d, i, c, b] = xc[b, g[i,c], d] (0 where invalid)
        xsch = xc[:, gc, :]  # [2, NM, C, D]
        xsch = np.where(valid[None, :, :, None], xsch, 0.0)
        xsch = np.ascontiguousarray(
            np.transpose(xsch, (3, 1, 2, 0)).reshape(D, NM, 2, CC // 2), dtype=ndt
        )
        m = {"xs": xsch, "wx": wx_c, "wh": wh_c}
        if has_bias:
            m["bias"] = np.ascontiguousarray(b.reshape(1, H), dtype=ndt)
        if has_h0:
            s0 = state0[BLOCAL * core : BLOCAL * (core + 1)]  # [2, H]
            corr = s0 @ Wh  # [2, H]
            m["hcorr"] = np.ascontiguousarray(
                np.transpose(corr.reshape(2, 2, 128), (2, 1, 0)), dtype=ndt
            )
        in_maps.append(m)

    res = run_bass_kernel_spmd(nc, in_maps, core_ids=list(range(NCORES)), trace=trace)
    LAST_EXEC_NS = res.exec_time_ns
    LAST_RESULTS = res

    outs = []
    for core in range(NCORES):
        yt = np.asarray(res.results[core]["yt"], dtype=np.float32)
        y = yt.reshape(128, 2, NM, C, 2)  # (p, kk, i, c, b); halves are
        # contiguous chunk ranges, so the flat order is unchanged
        y = np.transpose(y, (4, 3, 2, 1, 0))  # [2, C, NM, 2, 128]
        y = y[:, :, W:].reshape(BLOCAL, S, H)
        outs.append(y)
    return np.ascontiguousarray(np.concatenate(outs, axis=0), dtype=np.float32)


# revision 20
# speedup vs baseline: 1.1647x; 1.1647x over previous
import numpy as np

B, S, D, H = 16, 4096, 256, 256
NCORES = 8
BLOCAL = B // NCORES  # 2

_CACHE = {}


def _build(C=128, W=6, wdt_name="bfloat16", has_bias=False, has_h0=False):
    """Chunked-restart RNN scan, one core, B_local=2, two half-pipelines.

    The tanh recurrence forgets its state in ~32 steps (contractive), so each
    sequence is split into C chunks that advance in PARALLEL as matmul
    columns, with W warmup steps per chunk (chunk 0 exact). NM = S/C + W
    macro-steps replace S serial steps.

    The 2C columns are further split into two independent HALF-PIPELINES
    (chunks 0..C/2-1 and C/2..C-1) with separate PSUM banks and ht tiles:
    half X's matmuls execute while half Y's tanh runs, so the serial
    matmul->psum->tanh->sbuf round trip of one half hides behind the other.
    Per half and step: 4 quadrant matmuls ([128,128] lhsT x [128,HC] rhs,
    accumulating on the prefilled x-projection) and one Tanh [128, 2, HC].
    The xp GEMM for block n+1 is emitted piecewise before each step's scan
    matmuls (ready at emission, it fills tanh-idle PE slots).
    """
    import concourse.bass as bass
    import concourse.tile as tile
    from concourse import bacc, mybir

    f32 = mybir.dt.float32
    wdt = getattr(mybir.dt, wdt_name)
    L = S // C
    NM = L + W
    CC = 2 * C
    HC = CC // 2  # columns per half-pipeline
    SB = 256 // HC  # macro-steps per block: [128,2,SB,HC] f32 = one 2KB bank
    assert SB >= 1 and NM % SB == 0 and W % SB == 0
    NBLK = NM // SB
    Tanh = mybir.ActivationFunctionType.Tanh
    PSUM = bass.MemorySpace.PSUM

    nc = bacc.Bacc("TRN2", target_bir_lowering=False, debug=False)
    xs_d = nc.dram_tensor("xs", [D, NM, 2, HC], wdt, kind="ExternalInput")
    wx_d = nc.dram_tensor("wx", [D, H], wdt, kind="ExternalInput")
    wh_d = nc.dram_tensor("wh", [H, H], wdt, kind="ExternalInput")
    if has_bias:
        bias_d = nc.dram_tensor("bias", [1, H], wdt, kind="ExternalInput")
    if has_h0:
        hcorr_d = nc.dram_tensor("hcorr", [128, 2, 2], wdt, kind="ExternalInput")
    yt_d = nc.dram_tensor("yt", [128, 2, NM, 2, HC], wdt, kind="ExternalOutput")

    with tile.TileContext(nc) as tc:
        frees = []

        def T(shape, dt, name, space=None):
            kw = {"space": space} if space is not None else {}
            t, f = tc.tile(shape, dt, name=name, **kw)
            frees.append(f)
            return t

        wx_sb = T([128, 2, H], wdt, "wx_sb")
        scr_sb = T([128, 2], wdt, "scr_sb")
        wh_sb = T([128, 2, H], wdt, "wh_sb")
        h00_sb = T([128, 2, 2, HC], wdt, "h00_sb")
        xs_sb = [T([128, 2, SB, 2, HC], wdt, f"xs{i}") for i in range(3)]
        ht_sb = [
            [T([128, 2, SB, HC], wdt, f"ht{r}a{a}") for a in range(2)]
            for r in range(4)
        ]
        banks = [
            [T([128, 2, SB, HC], f32, f"pb{r}a{a}", space=PSUM) for a in range(2)]
            for r in range(4)
        ]
        if has_bias:
            bias_sb = T([1, H], wdt, "bias_sb")
            ones_sb = T([1, SB, 2, HC], wdt, "ones_sb")
            nc.sync.dma_start(bias_sb[:, :], bias_d[:, :])
        if has_h0:
            hcorr_sb = T([128, 2, 2], wdt, "hcorr_sb")
            nc.sync.dma_start(hcorr_sb[:, :, :], hcorr_d[:, :, :])

        def dma_in(blk, engines=None):
            if engines is None:
                engines = [nc.sync]
            for k in range(2):
                eng = engines[k % len(engines)]
                eng.dma_start(
                    xs_sb[blk % 3][:, k, :, :, :],
                    xs_d[k * 128 : (k + 1) * 128, blk * SB : (blk + 1) * SB, :, :],
                )

        def ones_for(blk):
            # bias applies everywhere except chunk 0's warmup columns (must
            # stay exactly zero; chunk 0 lives in half 0, column 0/1)
            nc.gpsimd.memset(ones_sb[:, :, :, :], 1.0)
            if blk * SB < W:
                nc.gpsimd.memset(ones_sb[0:1, :, 0, 0:2], 0.0)

        def gemm_mm(blk, ha, g):
            # g-th GEMM piece (of 4: m0k0, m0k1, m1k0, m1k1) prefilling block
            # blk's xp for half ha. m0k0 opens the bank's accumulation group
            # (start=True wipes the whole bank, so it must come first).
            m, k = g >> 1, g & 1
            nc.tensor.matmul(
                banks[blk % 4][ha][:, m, :, :],
                wx_sb[:, k, m * 128 : (m + 1) * 128],
                xs_sb[blk % 3][:, k, :, ha, :],
                start=(g == 0),
                stop=False,
                skip_group_check=True,
            )
            if has_bias and k == 1:
                nc.tensor.matmul(
                    banks[blk % 4][ha][:, m, :, :],
                    bias_sb[:, m * 128 : (m + 1) * 128],
                    ones_sb[:, :, ha, :],
                    start=False,
                    stop=False,
                    skip_group_check=True,
                )

        # prologue: block-0/1 xs first (gates the first GEMM pieces), then
        # weights split across queues; block 0's GEMM is emitted just-in-time
        # inside its first step below.
        # Scalar queue does ONLY the tanh-table preload (1283ns) so the
        # first real TANH doesn't pay it; weights go on sync/gpsimd after
        # the block-0 xs DMAs that gate the first GEMM pieces.
        nc.scalar.memzero(scr_sb[:, :])
        nc.scalar.activation(
            scr_sb[:, :], scr_sb[:, :], Tanh, bias=0.0, scale=1.0
        )
        dma_in(0, engines=[nc.sync, nc.gpsimd])
        nc.sync.dma_start(wx_sb[:, 0, :], wx_d[0:128, :])
        nc.sync.dma_start(wx_sb[:, 1, :], wx_d[128:256, :])
        nc.gpsimd.dma_start(wh_sb[:, 0, :], wh_d[0:128, :])
        nc.gpsimd.dma_start(wh_sb[:, 1, :], wh_d[128:256, :])
        dma_in(1)
        nc.gpsimd.memset(h00_sb[:, :, :, :], 0.0)
        if has_bias:
            ones_for(0)

        for blk in range(NBLK):
            hb = blk % 4
            if blk + 2 < NBLK and blk > 0:
                dma_in(blk + 2)
            if has_bias and blk + 1 < NBLK:
                ones_for(blk + 1)
            for j in range(SB):
                i = blk * SB + j
                for ha in range(2):
                    if blk == 0 and j == 0:
                        for g in range(4):
                            gemm_mm(0, ha, g)
                        if ha == 1 and NBLK > 2:
                            dma_in(2)  # all block-0 xs reads emitted by now
                    # next block's GEMM pieces: ready at emission, they run
                    # inside the other half's tanh window and keep PE warm
                    if blk + 1 < NBLK:
                        for g in range((4 * j) // SB, (4 * (j + 1)) // SB):
                            gemm_mm(blk + 1, ha, g)
                    if j > 0:
                        hp = ht_sb[hb][ha]
                        pj = j - 1
                    elif blk > 0:
                        hp = ht_sb[(hb - 1) % 4][ha]
                        pj = SB - 1
                    else:
                        hp = None
                    for k in range(2):
                        for m in range(2):
                            nc.tensor.matmul(
                                banks[hb][ha][:, m, j, :],
                                wh_sb[:, k, m * 128 : (m + 1) * 128],
                                h00_sb[:, k, ha, :] if hp is None else hp[:, k, pj, :],
                                start=False,
                                stop=(k == 1 and j == SB - 1),
                                skip_group_check=True,
                            )
                    if has_h0 and i == W and ha == 0:
                        # inject state0 @ Wh into chunk 0's first real column
                        for m in range(2):
                            for k in range(2):
                                nc.tensor.matmul(
                                    banks[hb][0][:, m, j, 0:2],
                                    wh_sb[:, k, m * 128 : (m + 1) * 128],
                                    hcorr_sb[:, k, :],
                                    start=False,
                                    stop=False,
                                    skip_group_check=True,
                                )
                    nc.scalar.activation(
                        ht_sb[hb][ha][:, :, j, :],
                        banks[hb][ha][:, :, j, :],
                        Tanh,
                        bias=0.0,
                        scale=1.0,
                    )
            if (blk + 1) * SB > W:  # warmup-only blocks are never read back
                for ha in range(2):
                    nc.gpsimd.dma_start(
                        yt_d[:, :, blk * SB : (blk + 1) * SB, ha, :],
                        ht_sb[hb][ha][:, :, :, :],
                    )

        for f in reversed(frees):
            f()

    nc.compile()
    return nc


def _get_nc(C, W, wdt_name, has_bias, has_h0):
    key = (C, W, wdt_name, has_bias, has_h0)
    if key not in _CACHE:
        _CACHE[key] = _build(C, W, wdt_name, has_bias, has_h0)
    return _CACHE[key]


LAST_EXEC_NS = None
LAST_RESULTS = None


def _np_dt(wdt_name):
    if wdt_name == "bfloat16":
        import ml_dtypes

        return ml_dtypes.bfloat16
    if wdt_name == "float16":
        return np.float16
    return np.float32


def kernel(inputs, state0, Wx, Wh, b, C=32, W=64, wdt_name="bfloat16", trace=False):
    global LAST_EXEC_NS, LAST_RESULTS
    from concourse.bass_utils import run_bass_kernel_spmd

    inputs = np.asarray(inputs, dtype=np.float32)
    state0 = np.asarray(state0, dtype=np.float32)
    Wx = np.asarray(Wx, dtype=np.float32)
    Wh = np.asarray(Wh, dtype=np.float32)
    b = np.asarray(b, dtype=np.float32)
    has_bias = bool(np.any(b != 0))
    has_h0 = bool(np.any(state0 != 0))
    ndt = _np_dt(wdt_name)
    L = S // C
    NM = L + W
    CC = 2 * C

    nc = _get_nc(C, W, wdt_name, has_bias, has_h0)

    wx_c = np.ascontiguousarray(Wx, dtype=ndt)
    wh_c = np.ascontiguousarray(Wh, dtype=ndt)

    # schedule gather indices: macro i, chunk c -> global step c*L - W + i
    ii = np.arange(NM)[:, None]
    cc_ = np.arange(C)[None, :]
    g = cc_ * L - W + ii  # [NM, C]
    valid = g >= 0
    gc = np.clip(g, 0, S - 1)

    in_maps = []
    for core in range(NCORES):
        xc = inputs[BLOCAL * core : BLOCAL * (core + 1)]  # [2, S, D]
        # xsched[d, i, c, b] = xc[b, g[i,c], d] (0 where invalid)
        xsch = xc[:, gc, :]  # [2, NM, C, D]
        xsch = np.where(valid[None, :, :, None], xsch, 0.0)
        xsch = np.ascontiguousarray(
            np.transpose(xsch, (3, 1, 2, 0)).reshape(D, NM, 2, CC // 2), dtype=ndt
        )
        m = {"xs": xsch, "wx": wx_c, "wh": wh_c}
        if has_bias:
            m["bias"] = np.ascontiguousarray(b.reshape(1, H), dtype=ndt)
        if has_h0:
            s0 = state0[BLOCAL * core : BLOCAL * (core + 1)]  # [2, H]
            corr = s0 @ Wh  # [2, H]
            m["hcorr"] = np.ascontiguousarray(
                np.transpose(corr.reshape(2, 2, 128), (2, 1, 0)), dtype=ndt
            )
        in_maps.append(m)

    res = run_bass_kernel_spmd(nc, in_maps, core_ids=list(range(NCORES)), trace=trace)
    LAST_EXEC_NS = res.exec_time_ns
    LAST_RESULTS = res

    outs = []
    for core in range(NCORES):
        yt = np.asarray(res.results[core]["yt"], dtype=np.float32)
        y = yt.reshape(128, 2, NM, C, 2)  # (p, kk, i, c, b); halves are
        # contiguous chunk ranges, so the flat order is unchanged
        y = np.transpose(y, (4, 3, 2, 1, 0))  # [2, C, NM, 2, 128]
        y = y[:, :, W:].reshape(BLOCAL, S, H)
        outs.append(y)
    return np.ascontiguousarray(np.concatenate(outs, axis=0), dtype=np.float32)
